# revision 5
# baseline (speedup 1.0000x reference)
"""Self-contained Trainium2 Bass kernel for single-head causal attention (v2).

reference math (per batch element b):
    Q = x @ Wq + bq ; K = x @ Wk + bk ; V = x @ Wv + bv          [S, H]
    wei = Q @ K^T  (no 1/sqrt(d) scaling)                        [S, S]
    wei = tril-masked, exact-zeros -> -inf (no-op for this data)
    attn = softmax(wei) * drop_mask
    out = attn @ V                                               [S, H]

Device strategy (one NeuronCore per batch element, 8 cores):
  - x^T travels as fp16 (halves HBM traffic; |x| < 6 so fp16's 11-bit
    mantissa loses less than the f32r matmul itself), weights as packed
    fp16 [D, 3H] in one DMA, drop_mask^T tile-packed on host into one
    [128, 17408] bf16 buffer covering only the causal tiles, loaded in
    4 superchunk DMAs (vs 40 per-tile DMAs: each HWDGE DMA costs ~625ns
    serialized).
  - projections: fp16 matmuls into PSUM f32; bias-add + copy-out happens
    on the Pool engine (tensor_scalar add) to keep Act/DVE free.
  - scores E^T = exp(K^T_t q) in [t, s] layout (f32r matmuls on qt/kt);
    causal masking of diagonal tiles via an extra accumulation matmul
    (trilT^T @ I adds -1e30 above the diagonal) instead of a DVE pass.
  - e = exp(scores) in bf16 (Act); p = e * mask in bf16 (DVE, all-16-bit
    2x mode); softmax denominator accumulated as es += e in bf16 on DVE,
    then per-128-block column rowsums via tiny es^T@ones matmuls (PSUM
    columns directly - no transposes or staging).
  - out^T accumulated in PSUM over t-tiles (v_sb bf16 stationary x p);
    per-superchunk epilogue: PSUM->SBUF copy (Pool), PE transpose back,
    scale by 1/rowsum (DVE reciprocal + one Newton step) on the Act
    copy-out, DMA [S, H] f32 to HBM.
  - projections of chunk c+1 are interleaved into attention superchunk c
    so proj matmuls fill PE pipeline bubbles while Act/DVE catch up.
"""

import contextlib
import os
import sys

os.environ.setdefault("MYCRO_LOCAL_CACHE", "1")
for _p in ("/opt/trn_rl_repo",):
    if _p not in sys.path:
        sys.path.insert(0, _p)

import ml_dtypes
import numpy as np

import concourse.bacc as bacc
import concourse.tile as tile
from concourse import mybir
from concourse.bass import ds, ts
from concourse.bass_utils import run_bass_kernel_spmd

AF = mybir.ActivationFunctionType
ALU = mybir.AluOpType
F32 = mybir.dt.float32
F32R = mybir.dt.float32r
BF16 = mybir.dt.bfloat16
FP16 = mybir.dt.float16

B, S, D, H = 8, 2048, 1024, 128
NCORES = 8
SCW = 512  # s-superchunk width (one PSUM bank of f32)
NEG = -1.0e30


def _mask_layout(s=S):
    """Causal tile list per superchunk: (i, c0, n, off) with off = column
    offset into the host-packed [128, MW] mask buffer."""
    n_sc = s // SCW
    tpc = SCW // 128
    tiles = []
    off = 0
    for sc in range(n_sc):
        row = []
        for i in range(tpc * sc + tpc):
            c0 = max(0, 128 * i - SCW * sc)
            n = SCW - c0
            row.append((i, c0, n, off))
            off += n
        tiles.append(row)
    return tiles, off


def build_nc(s=S, d=D, h=H, num_devices=NCORES, reps=1):
    assert h == 128 and s % SCW == 0 and d % 128 == 0
    n_sc = s // SCW
    n_k = d // 128
    tpc = SCW // 128
    kh = n_k // 2
    tiles_by_sc, MW = _mask_layout(s)

    nc = bacc.Bacc(
        "TRN2", target_bir_lowering=False, debug=False, num_devices=num_devices
    )

    xt_d = nc.dram_tensor("xt", [d, s], FP16, kind="ExternalInput")
    w_d = nc.dram_tensor("wqkv", [d, 3 * h], FP16, kind="ExternalInput")
    b_d = nc.dram_tensor("b3", [h, 3], F32, kind="ExternalInput")
    mask_d = nc.dram_tensor("maskp", [128, MW], BF16, kind="ExternalInput")
    identb_d = nc.dram_tensor("identb", [128, 128], BF16, kind="ExternalInput")
    identr_d = nc.dram_tensor("identr", [128, 128], F32R, kind="ExternalInput")
    trilt_d = nc.dram_tensor("trilt", [128, 128], BF16, kind="ExternalInput")
    onesb_d = nc.dram_tensor("onesb", [128, 1], BF16, kind="ExternalInput")
    out_d = nc.dram_tensor("out", [s, h], F32, kind="ExternalOutput")

    xt3 = xt_d.rearrange("(k p) s -> p k s", p=128)
    w4 = w_d.rearrange("(k p) (m h) -> p k m h", p=128, m=3)

    with tile.TileContext(nc) as tc:
        with (
            tc.tile_pool(name="consts", bufs=1) as consts,
            tc.tile_pool(name="xt", bufs=1) as xtp,
            tc.tile_pool(name="proj", bufs=1) as projp,
            tc.tile_pool(name="vt", bufs=2) as vtp,
            tc.tile_pool(name="mask", bufs=1) as maskp,
            tc.tile_pool(name="e", bufs=7) as ep,
            tc.tile_pool(name="p", bufs=6) as pp,
            tc.tile_pool(name="es", bufs=2) as esp,
            tc.tile_pool(name="otsb", bufs=2) as otp,
            tc.tile_pool(name="osc", bufs=2) as oscp,
            tc.tile_pool(name="small", bufs=8) as smallp,
            tc.tile_pool(name="ps_proj", bufs=2, space="PSUM") as ps_proj,
            tc.tile_pool(name="ps_sc", bufs=4, space="PSUM") as ps_sc,
            tc.tile_pool(name="ps_ot", bufs=1, space="PSUM") as ps_ot,
            tc.tile_pool(name="ps_tr", bufs=1, space="PSUM") as ps_tr,
        ):
            # ---- persistent tiles ----
            w_sb = consts.tile([128, n_k, 3, h], FP16, tag="w")
            b_sb = consts.tile([h, 3], F32, tag="b")
            identb = consts.tile([128, 128], BF16, tag="identb")
            identr = consts.tile([128, 128], F32R, tag="identr")
            trilt = consts.tile([128, 128], BF16, tag="trilt")
            onesb = consts.tile([128, 1], BF16, tag="onesb")

            xt = xtp.tile([128, n_k, s], FP16, tag="xt")
            qt = projp.tile([h, s], F32R, tag="qt")
            kt = projp.tile([h, s], F32R, tag="kt")
            v_sb = projp.tile([128, s], BF16, tag="v")  # col block i = V tile i
            msk = maskp.tile([128, MW], BF16, tag="msk")

            # ---- weight + const loads (outside the reps loop) ----
            # order matters: the first proj matmuls need xt[k<2]+w[k<4] -
            # those are issued from inside the loop body, so keep the big
            # const DMAs fine-grained and early-consumer-first.
            nc.sync.dma_start(w_sb[:, 0:kh], w4[:, 0:kh])
            nc.sync.dma_start(b_sb[:], b_d[:])
            nc.sync.dma_start(identb[:], identb_d[:])
            nc.sync.dma_start(trilt[:], trilt_d[:])
            nc.sync.dma_start(w_sb[:, kh:n_k], w4[:, kh:n_k])
            nc.sync.dma_start(identr[:], identr_d[:])
            nc.sync.dma_start(onesb[:], onesb_d[:])

            def load_xt(c, nsplit=2):
                step = n_k // nsplit
                for k0 in range(0, n_k, step):
                    nc.sync.dma_start(
                        xt[:, k0 : k0 + step, ds(c * SCW, SCW)],
                        xt3[:, k0 : k0 + step, ds(c * SCW, SCW)],
                    )

            def load_mask(sc):
                row = tiles_by_sc[sc]
                off0 = row[0][3]
                w = sum(t[2] for t in row)
                # split wide loads so early tiles' mask lands sooner
                half = (w // 2 + 127) & ~127 if w > 4096 else w
                for o in range(0, w, half):
                    step = min(half, w - o)
                    nc.sync.dma_start(
                        msk[:, off0 + o : off0 + o + step],
                        mask_d[:, off0 + o : off0 + o + step],
                    )

            def proj_units(c, kstep=2):
                """Emission units for projecting chunk c (q, k, v), split
                into kstep-sized matmul groups so they can fill PE bubbles
                in the interleaved attention superchunk."""
                chunk = ds(c * SCW, SCW)
                units = []
                for m in range(3):
                    box = {}
                    for k0 in range(0, n_k, kstep):

                        def u(m=m, box=box, k0=k0):
                            if k0 == 0:
                                box["ps"] = ps_proj.tile(
                                    [128, SCW], F32, tag="pp", name="pp"
                                )
                            pst = box["ps"]
                            for k in range(k0, k0 + kstep):
                                nc.tensor.matmul(
                                    pst[:],
                                    w_sb[:, k, m, :],
                                    xt[:, k, chunk],
                                    start=(k == 0),
                                    stop=(k == n_k - 1),
                                    skip_group_check=True,
                                )

                        units.append(u)

                    def ub(m=m, box=box):
                        # bias-add + PSUM->SBUF copy on Act (Pool cannot
                        # read PSUM on TRN2 hardware)
                        pst = box["ps"]
                        if m == 0:
                            nc.scalar.activation(
                                qt[:, chunk], pst[:], AF.Identity,
                                bias=b_sb[:, 0:1],
                            )
                        elif m == 1:
                            nc.scalar.activation(
                                kt[:, chunk], pst[:], AF.Identity,
                                bias=b_sb[:, 1:2],
                            )
                        else:
                            vt = vtp.tile([128, SCW], BF16, tag="vt")
                            box["vt"] = vt
                            nc.scalar.activation(
                                vt[:], pst[:], AF.Identity,
                                bias=b_sb[:, 2:3],
                            )

                    units.append(ub)
                    if m == 2:

                        def u3(box=box):
                            vt = box["vt"]
                            tp = ps_tr.tile([128, SCW], BF16, tag="tr", name="tp")
                            for qq in range(tpc):
                                nc.tensor.transpose(
                                    tp[:, ts(qq, 128)], vt[:, ts(qq, 128)],
                                    identb[:],
                                )
                            nc.vector.tensor_copy(v_sb[:, chunk], tp[:])

                        units.append(u3)
                return units

            def emit_attn(sc, units):
                """Attention superchunk sc; pops interleave `units` between
                tiles to keep PE fed while Act/DVE catch up."""
                row = tiles_by_sc[sc]
                nt = len(row)
                es = esp.tile([128, SCW], BF16, tag="es")
                ot_ps = ps_ot.tile([128, SCW], F32, tag="ot")
                pend_av = []
                pend_es = []
                e_tiles = []
                SKEW = 4
                ESKEW = 2

                for idx, (i, c0, n, off) in enumerate(row):
                    diag = i >= tpc * sc
                    scp = ps_sc.tile([128, n], F32, tag="sc")
                    nc.tensor.matmul(
                        scp[:],
                        kt[:, ts(i, 128)],
                        qt[:, ds(SCW * sc + c0, n)],
                        start=True,
                        stop=not diag,
                        skip_group_check=True,
                    )
                    if diag:
                        nc.tensor.matmul(
                            scp[:, 0:128],
                            trilt[:],
                            identb[:],
                            start=False,
                            stop=True,
                            skip_group_check=True,
                        )
                    e = ep.tile([128, n], BF16, tag="e")
                    nc.scalar.activation(e[:], scp[:], AF.Exp)
                    p = pp.tile([128, n], BF16, tag="p")
                    nc.vector.tensor_tensor(
                        p[:], e[:], msk[:, off : off + n], op=ALU.mult
                    )
                    e_tiles.append(e)

                    def esadd(idx=idx, c0=c0, e=e, e_prev=(row[0], None)):
                        with nc.allow_low_precision("bf16 denom accum"):
                            if idx == 0:
                                if sc > 0:
                                    return  # folded into idx 1's paired add
                                nc.vector.tensor_copy(es[:], e[:])
                            elif idx == 1 and sc > 0:
                                nc.vector.tensor_tensor(
                                    es[:], e_tiles[0], e[:], op=ALU.add
                                )
                            else:
                                nc.vector.tensor_tensor(
                                    es[:, c0:SCW], es[:, c0:SCW], e[:],
                                    op=ALU.add,
                                )

                    def av(idx=idx, i=i, c0=c0, n=n, p=p):
                        nc.tensor.matmul(
                            ot_ps[:, ds(c0, n)],
                            v_sb[:, ts(i, 128)],
                            p[:],
                            start=(idx == 0),
                            stop=(idx == nt - 1),
                            skip_group_check=True,
                        )

                    pend_av.append(av)
                    pend_es.append(esadd)
                    if idx >= SKEW:
                        pend_av[idx - SKEW]()
                    if idx >= ESKEW:
                        pend_es[idx - ESKEW]()
                    if units:
                        units.pop(0)()

                for av in pend_av[max(0, nt - SKEW) :]:
                    av()
                for esadd in pend_es[max(0, nt - ESKEW) :]:
                    esadd()
                while units:
                    units.pop(0)()

                # ---- per-superchunk epilogue ----
                rs = ps_tr.tile([128, tpc], F32, tag="tr", name="rs")
                for qq in range(tpc):
                    nc.tensor.matmul(
                        rs[:, qq : qq + 1],
                        es[:, ts(qq, 128)],
                        onesb[:],
                        start=True,
                        stop=True,
                        skip_group_check=True,
                    )
                r0 = smallp.tile([128, tpc], F32, tag="r0")
                nc.vector.reciprocal(r0[:], rs[:])
                t1 = smallp.tile([128, tpc], F32, tag="t1")
                nc.vector.tensor_tensor(t1[:], rs[:], r0[:], op=ALU.mult)
                t2 = smallp.tile([128, tpc], F32, tag="t2")
                nc.vector.tensor_scalar(
                    t2[:], t1[:], -1.0, 2.0, op0=ALU.mult, op1=ALU.add
                )
                r1 = smallp.tile([128, tpc], F32, tag="r1")
                nc.vector.tensor_tensor(r1[:], r0[:], t2[:], op=ALU.mult)

                ot_sb = otp.tile([128, SCW], BF16, tag="otsb")
                with nc.allow_low_precision("bf16 out staging"):
                    nc.scalar.activation(ot_sb[:], ot_ps[:], AF.Copy)
                ott = ps_tr.tile([128, SCW], BF16, tag="tr", name="ott")
                for qq in range(tpc):
                    nc.tensor.transpose(
                        ott[:, ts(qq, 128)], ot_sb[:, ts(qq, 128)], identb[:]
                    )
                osc = oscp.tile([128, tpc, h], F32, tag="osc")
                for qq in range(tpc):
                    nc.vector.tensor_scalar(
                        osc[:, qq, :],
                        ott[:, ts(qq, 128)],
                        r1[:, qq : qq + 1],
                        None,
                        op0=ALU.mult,
                    )
                out_view = out_d[ds(SCW * sc, SCW), :].rearrange(
                    "(q p) h -> p q h", p=128
                )
                nc.sync.dma_start(out_view, osc[:])

            def emit_body(preload_next):
                """One full iteration; assumes xt chunk 0 + mask sc 0 are
                already loaded (prologue or previous body's preload)."""
                for u in proj_units(0):
                    u()
                for sc in range(n_sc):
                    units = []
                    if sc + 1 < n_sc:
                        units.append(lambda c=sc + 1: load_xt(c, nsplit=2))
                        units += proj_units(sc + 1)
                        units.append(lambda sc2=sc + 1: load_mask(sc2))
                    elif preload_next:
                        units.append(lambda: load_xt(0, nsplit=2))
                        units.append(lambda: load_mask(0))
                    emit_attn(sc, units)

            # software pipeline: chunk-0 data for the first body
            load_xt(0)
            load_mask(0)
            if reps == 1:
                emit_body(preload_next=False)
            else:
                unroll = next(
                    (u for u in (8, 4, 2) if reps % u == 0), 1
                )
                with tc.For_i(0, reps // unroll, 1):
                    for _ in range(unroll):
                        emit_body(preload_next=True)

    nc.compile()
    return nc


def host_inputs(input, Wq, bq, Wk, bk, Wv, bv, drop_mask):
    """Build the per-core in_maps from the full problem inputs."""
    tiles_by_sc, MW = _mask_layout(S)
    idx = np.arange(128)
    # M[t, s] = 0 if t <= s else NEG ; matmul adds trilt[s, t] so ship M^T
    m_ts = np.where(idx[:, None] <= idx[None, :], 0.0, NEG).astype(np.float32)
    trilt = np.ascontiguousarray(m_ts.T).astype(ml_dtypes.bfloat16)
    shared = {
        "wqkv": np.ascontiguousarray(
            np.concatenate([Wq, Wk, Wv], axis=1).astype(np.float16)
        ),
        "b3": np.ascontiguousarray(
            np.stack(
                [np.asarray(bq), np.asarray(bk), np.asarray(bv)], axis=1
            ).astype(np.float32)
        ),
        "identb": np.eye(128, dtype=ml_dtypes.bfloat16),
        "identr": np.eye(128, dtype=np.float32),
        "trilt": trilt,
        "onesb": np.ones((128, 1), ml_dtypes.bfloat16),
    }
    in_maps = []
    for b in range(B):
        mt = np.asarray(drop_mask[b], np.float32).T  # [t, s]
        blocks = []
        for sc, row in enumerate(tiles_by_sc):
            for i, c0, n, off in row:
                scol = SCW * sc + c0
                blocks.append(mt[128 * i : 128 * (i + 1), scol : scol + n])
        maskp = np.ascontiguousarray(np.concatenate(blocks, axis=1)).astype(
            ml_dtypes.bfloat16
        )
        assert maskp.shape == (128, MW)
        in_maps.append(
            dict(
                shared,
                xt=np.ascontiguousarray(
                    np.asarray(input[b], np.float32).T.astype(np.float16)
                ),
                maskp=maskp,
            )
        )
    return in_maps


_NC_CACHE = {}


def get_nc(**kw):
    key = tuple(sorted(kw.items()))
    if key not in _NC_CACHE:
        _NC_CACHE[key] = build_nc(**kw)
    return _NC_CACHE[key]


def kernel(input, Wq, bq, Wk, bk, Wv, bv, drop_mask, **run_kwargs):
    nc = get_nc()
    in_maps = host_inputs(input, Wq, bq, Wk, bk, Wv, bv, drop_mask)
    res = run_bass_kernel_spmd(
        nc, in_maps, core_ids=list(range(NCORES)), **run_kwargs
    )
    out = np.stack([r["out"] for r in res.results]).astype(np.float32)
    if run_kwargs:
        kernel.last_result = res
    return out


# revision 6
# speedup vs baseline: 1.0225x; 1.0225x over previous
"""Self-contained Trainium2 Bass kernel for single-head causal attention (v2).

reference math (per batch element b):
    Q = x @ Wq + bq ; K = x @ Wk + bk ; V = x @ Wv + bv          [S, H]
    wei = Q @ K^T  (no 1/sqrt(d) scaling)                        [S, S]
    wei = tril-masked, exact-zeros -> -inf (no-op for this data)
    attn = softmax(wei) * drop_mask
    out = attn @ V                                               [S, H]

Device strategy (one NeuronCore per batch element, 8 cores):
  - x^T travels as fp16 (halves HBM traffic; |x| < 6 so fp16's 11-bit
    mantissa loses less than the f32r matmul itself), weights as packed
    fp16 [D, 3H] in one DMA, drop_mask^T tile-packed on host into one
    [128, 17408] bf16 buffer covering only the causal tiles, loaded in
    4 superchunk DMAs (vs 40 per-tile DMAs: each HWDGE DMA costs ~625ns
    serialized).
  - projections: fp16 matmuls into PSUM f32; bias-add + copy-out on the
    Act engine (GPSIMD/Pool cannot touch PSUM on TRN2 hardware).
  - scores E^T = exp(K^T_t q) in [t, s] layout (f32r matmuls on qt/kt);
    causal masking of diagonal tiles via an extra accumulation matmul
    (trilT^T @ I adds -1e30 above the diagonal) instead of a DVE pass.
  - e = exp(scores) in bf16 (Act); p = e * mask in bf16 (DVE, all-16-bit
    2x mode); softmax denominator accumulated as es += e in bf16 on DVE,
    then per-128-block column rowsums via tiny es^T@ones matmuls (PSUM
    columns directly - no transposes or staging).
  - out^T accumulated in PSUM over t-tiles (v_sb bf16 stationary x p);
    per-superchunk epilogue: PSUM->SBUF bf16 copy (Act), PE transpose
    back, scale by 1/rowsum (DVE reciprocal + one Newton step) on the
    DVE copy-out, DMA [S, H] f32 to HBM. The whole body sits in a 4-8x
    unrolled For_i with next-iteration chunk-0 x/mask preloaded, so the
    per-iteration barrier never stalls the front of the pipeline.
  - projections of chunk c+1 are interleaved into attention superchunk c
    so proj matmuls fill PE pipeline bubbles while Act/DVE catch up.
"""

import contextlib
import os
import sys

os.environ.setdefault("MYCRO_LOCAL_CACHE", "1")
for _p in ("/opt/trn_rl_repo",):
    if _p not in sys.path:
        sys.path.insert(0, _p)

import ml_dtypes
import numpy as np

import concourse.bacc as bacc
import concourse.tile as tile
from concourse import mybir
from concourse.bass import ds, ts
from concourse.bass_utils import run_bass_kernel_spmd

AF = mybir.ActivationFunctionType
ALU = mybir.AluOpType
F32 = mybir.dt.float32
F32R = mybir.dt.float32r
BF16 = mybir.dt.bfloat16
FP16 = mybir.dt.float16

B, S, D, H = 8, 2048, 1024, 128
NCORES = 8
SCW = 512  # s-superchunk width (one PSUM bank of f32)
NEG = -1.0e30


def _mask_layout(s=S):
    """Causal tile list per superchunk: (i, c0, n, off) with off = column
    offset into the host-packed [128, MW] mask buffer."""
    n_sc = s // SCW
    tpc = SCW // 128
    tiles = []
    off = 0
    for sc in range(n_sc):
        row = []
        for i in range(tpc * sc + tpc):
            c0 = max(0, 128 * i - SCW * sc)
            n = SCW - c0
            row.append((i, c0, n, off))
            off += n
        tiles.append(row)
    return tiles, off


def build_nc(s=S, d=D, h=H, num_devices=NCORES, reps=1):
    assert h == 128 and s % SCW == 0 and d % 128 == 0
    n_sc = s // SCW
    n_k = d // 128
    tpc = SCW // 128
    kh = n_k // 2
    tiles_by_sc, MW = _mask_layout(s)

    nc = bacc.Bacc(
        "TRN2", target_bir_lowering=False, debug=False, num_devices=num_devices
    )

    xt_d = nc.dram_tensor("xt", [d, s], FP16, kind="ExternalInput")
    w_d = nc.dram_tensor("wqkv", [d, 3 * h], FP16, kind="ExternalInput")
    b_d = nc.dram_tensor("b3", [h, 3], F32, kind="ExternalInput")
    mask_d = nc.dram_tensor("maskp", [128, MW], BF16, kind="ExternalInput")
    identb_d = nc.dram_tensor("identb", [128, 128], BF16, kind="ExternalInput")
    identr_d = nc.dram_tensor("identr", [128, 128], F32R, kind="ExternalInput")
    trilt_d = nc.dram_tensor("trilt", [128, 128], BF16, kind="ExternalInput")
    onesb_d = nc.dram_tensor("onesb", [128, 1], BF16, kind="ExternalInput")
    out_d = nc.dram_tensor("out", [s, h], F32, kind="ExternalOutput")

    xt3 = xt_d.rearrange("(k p) s -> p k s", p=128)
    w4 = w_d.rearrange("(k p) (m h) -> p k m h", p=128, m=3)

    with tile.TileContext(nc) as tc:
        with (
            tc.tile_pool(name="consts", bufs=1) as consts,
            tc.tile_pool(name="xt", bufs=1) as xtp,
            tc.tile_pool(name="proj", bufs=1) as projp,
            tc.tile_pool(name="vt", bufs=2) as vtp,
            tc.tile_pool(name="mask", bufs=1) as maskp,
            tc.tile_pool(name="e", bufs=6) as ep,
            tc.tile_pool(name="p", bufs=5) as pp,
            tc.tile_pool(name="es", bufs=2) as esp,
            tc.tile_pool(name="otsb", bufs=2) as otp,
            tc.tile_pool(name="osc", bufs=2) as oscp,
            tc.tile_pool(name="small", bufs=8) as smallp,
            tc.tile_pool(name="ps_proj", bufs=2, space="PSUM") as ps_proj,
            tc.tile_pool(name="ps_sc", bufs=4, space="PSUM") as ps_sc,
            tc.tile_pool(name="ps_ot", bufs=1, space="PSUM") as ps_ot,
            tc.tile_pool(name="ps_tr", bufs=1, space="PSUM") as ps_tr,
        ):
            # ---- persistent tiles ----
            w_sb = consts.tile([128, n_k, 3, h], FP16, tag="w")
            b_sb = consts.tile([h, 3], F32, tag="b")
            identb = consts.tile([128, 128], BF16, tag="identb")
            identr = consts.tile([128, 128], F32R, tag="identr")
            trilt = consts.tile([128, 128], BF16, tag="trilt")
            onesb = consts.tile([128, 1], BF16, tag="onesb")

            xt = xtp.tile([128, n_k, s], FP16, tag="xt")
            qt = projp.tile([h, s], F32R, tag="qt")
            kt = projp.tile([h, s], F32R, tag="kt")
            v_sb = projp.tile([128, s], BF16, tag="v")  # col block i = V tile i
            msk = maskp.tile([128, MW], BF16, tag="msk")

            # ---- weight + const loads (outside the reps loop) ----
            # order matters: the first proj matmuls need xt[k<2]+w[k<4] -
            # those are issued from inside the loop body, so keep the big
            # const DMAs fine-grained and early-consumer-first.
            nc.sync.dma_start(w_sb[:, 0:kh], w4[:, 0:kh])
            nc.sync.dma_start(b_sb[:], b_d[:])
            nc.sync.dma_start(identb[:], identb_d[:])
            nc.sync.dma_start(trilt[:], trilt_d[:])
            nc.sync.dma_start(w_sb[:, kh:n_k], w4[:, kh:n_k])
            nc.sync.dma_start(identr[:], identr_d[:])
            nc.sync.dma_start(onesb[:], onesb_d[:])

            def load_xt(c, nsplit=2):
                step = n_k // nsplit
                for k0 in range(0, n_k, step):
                    nc.sync.dma_start(
                        xt[:, k0 : k0 + step, ds(c * SCW, SCW)],
                        xt3[:, k0 : k0 + step, ds(c * SCW, SCW)],
                    )

            def load_mask(sc):
                row = tiles_by_sc[sc]
                off0 = row[0][3]
                w = sum(t[2] for t in row)
                # split wide loads so early tiles' mask lands sooner
                half = (w // 2 + 127) & ~127 if w > 4096 else w
                for o in range(0, w, half):
                    step = min(half, w - o)
                    nc.sync.dma_start(
                        msk[:, off0 + o : off0 + o + step],
                        mask_d[:, off0 + o : off0 + o + step],
                    )

            def proj_units(c, kstep=2):
                """Emission units for projecting chunk c (q, k, v), split
                into kstep-sized matmul groups so they can fill PE bubbles
                in the interleaved attention superchunk."""
                chunk = ds(c * SCW, SCW)
                units = []
                for m in range(3):
                    box = {}
                    for k0 in range(0, n_k, kstep):

                        def u(m=m, box=box, k0=k0):
                            if k0 == 0:
                                box["ps"] = ps_proj.tile(
                                    [128, SCW], F32, tag="pp", name="pp"
                                )
                            pst = box["ps"]
                            for k in range(k0, k0 + kstep):
                                nc.tensor.matmul(
                                    pst[:],
                                    w_sb[:, k, m, :],
                                    xt[:, k, chunk],
                                    start=(k == 0),
                                    stop=(k == n_k - 1),
                                    skip_group_check=True,
                                )

                        units.append(u)

                    def ub(m=m, box=box):
                        # bias-add + PSUM->SBUF copy on Act (Pool cannot
                        # read PSUM on TRN2 hardware)
                        pst = box["ps"]
                        if m == 0:
                            nc.scalar.activation(
                                qt[:, chunk], pst[:], AF.Identity,
                                bias=b_sb[:, 0:1],
                            )
                        elif m == 1:
                            nc.scalar.activation(
                                kt[:, chunk], pst[:], AF.Identity,
                                bias=b_sb[:, 1:2],
                            )
                        else:
                            vt = vtp.tile([128, SCW], BF16, tag="vt")
                            box["vt"] = vt
                            nc.scalar.activation(
                                vt[:], pst[:], AF.Identity,
                                bias=b_sb[:, 2:3],
                            )

                    units.append(ub)
                    if m == 2:

                        def u3(box=box):
                            vt = box["vt"]
                            tp = ps_tr.tile([128, SCW], BF16, tag="tr", name="tp")
                            for qq in range(tpc):
                                nc.tensor.transpose(
                                    tp[:, ts(qq, 128)], vt[:, ts(qq, 128)],
                                    identb[:],
                                )
                            nc.scalar.activation(
                                v_sb[:, chunk], tp[:], AF.Copy
                            )

                        units.append(u3)
                return units

            def emit_attn(sc, units):
                """Attention superchunk sc; pops interleave `units` between
                tiles to keep PE fed while Act/DVE catch up."""
                row = tiles_by_sc[sc]
                nt = len(row)
                es = esp.tile([128, SCW], BF16, tag="es")
                ot_ps = ps_ot.tile([128, SCW], F32, tag="ot")
                pend_av = []
                pend_es = []
                e_tiles = []
                SKEW = 3
                ESKEW = 2

                for idx, (i, c0, n, off) in enumerate(row):
                    diag = i >= tpc * sc
                    scp = ps_sc.tile([128, n], F32, tag="sc")
                    nc.tensor.matmul(
                        scp[:],
                        kt[:, ts(i, 128)],
                        qt[:, ds(SCW * sc + c0, n)],
                        start=True,
                        stop=not diag,
                        skip_group_check=True,
                    )
                    if diag:
                        nc.tensor.matmul(
                            scp[:, 0:128],
                            trilt[:],
                            identb[:],
                            start=False,
                            stop=True,
                            skip_group_check=True,
                        )
                    e = ep.tile([128, n], BF16, tag="e")
                    nc.scalar.activation(e[:], scp[:], AF.Exp)
                    p = pp.tile([128, n], BF16, tag="p")
                    nc.vector.tensor_tensor(
                        p[:], e[:], msk[:, off : off + n], op=ALU.mult
                    )
                    e_tiles.append(e)

                    def esadd(idx=idx, c0=c0, e=e, e_prev=(row[0], None)):
                        with nc.allow_low_precision("bf16 denom accum"):
                            if idx == 0:
                                if sc > 0:
                                    return  # folded into idx 1's paired add
                                nc.vector.tensor_copy(es[:], e[:])
                            elif idx == 1 and sc > 0:
                                nc.vector.tensor_tensor(
                                    es[:], e_tiles[0], e[:], op=ALU.add
                                )
                            else:
                                nc.vector.tensor_tensor(
                                    es[:, c0:SCW], es[:, c0:SCW], e[:],
                                    op=ALU.add,
                                )

                    def av(idx=idx, i=i, c0=c0, n=n, p=p):
                        nc.tensor.matmul(
                            ot_ps[:, ds(c0, n)],
                            v_sb[:, ts(i, 128)],
                            p[:],
                            start=(idx == 0),
                            stop=(idx == nt - 1),
                            skip_group_check=True,
                        )

                    pend_av.append(av)
                    pend_es.append(esadd)
                    if idx >= SKEW:
                        pend_av[idx - SKEW]()
                    if idx >= ESKEW:
                        pend_es[idx - ESKEW]()
                    if units:
                        units.pop(0)()

                for av in pend_av[max(0, nt - SKEW) :]:
                    av()
                for esadd in pend_es[max(0, nt - ESKEW) :]:
                    esadd()
                while units:
                    units.pop(0)()

                # ---- per-superchunk epilogue ----
                rs = ps_tr.tile([128, tpc], F32, tag="tr", name="rs")
                for qq in range(tpc):
                    nc.tensor.matmul(
                        rs[:, qq : qq + 1],
                        es[:, ts(qq, 128)],
                        onesb[:],
                        start=True,
                        stop=True,
                        skip_group_check=True,
                    )
                r0 = smallp.tile([128, tpc], F32, tag="r0")
                nc.vector.reciprocal(r0[:], rs[:])
                t1 = smallp.tile([128, tpc], F32, tag="t1")
                nc.vector.tensor_tensor(t1[:], rs[:], r0[:], op=ALU.mult)
                t2 = smallp.tile([128, tpc], F32, tag="t2")
                nc.vector.tensor_scalar(
                    t2[:], t1[:], -1.0, 2.0, op0=ALU.mult, op1=ALU.add
                )
                r1 = smallp.tile([128, tpc], F32, tag="r1")
                nc.vector.tensor_tensor(r1[:], r0[:], t2[:], op=ALU.mult)

                ot_sb = otp.tile([128, SCW], BF16, tag="otsb")
                with nc.allow_low_precision("bf16 out staging"):
                    nc.scalar.activation(ot_sb[:], ot_ps[:], AF.Copy)
                ott = ps_tr.tile([128, SCW], BF16, tag="tr", name="ott")
                for qq in range(tpc):
                    nc.tensor.transpose(
                        ott[:, ts(qq, 128)], ot_sb[:, ts(qq, 128)], identb[:]
                    )
                osc = oscp.tile([128, tpc, h], F32, tag="osc")
                for qq in range(tpc):
                    nc.vector.tensor_scalar(
                        osc[:, qq, :],
                        ott[:, ts(qq, 128)],
                        r1[:, qq : qq + 1],
                        None,
                        op0=ALU.mult,
                    )
                out_view = out_d[ds(SCW * sc, SCW), :].rearrange(
                    "(q p) h -> p q h", p=128
                )
                nc.sync.dma_start(out_view, osc[:])

            def emit_body(preload_next):
                """One full iteration; assumes xt chunk 0 + mask sc 0 are
                already loaded (prologue or previous body's preload)."""
                for u in proj_units(0):
                    u()
                for sc in range(n_sc):
                    units = []
                    if sc + 1 < n_sc:
                        units.append(lambda c=sc + 1: load_xt(c, nsplit=2))
                        units += proj_units(sc + 1)
                        units.append(lambda sc2=sc + 1: load_mask(sc2))
                    elif preload_next:
                        units.append(lambda: load_xt(0, nsplit=2))
                        units.append(lambda: load_mask(0))
                    emit_attn(sc, units)

            # software pipeline: chunk-0 data for the first body
            load_xt(0)
            load_mask(0)
            if reps == 1:
                emit_body(preload_next=False)
            else:
                unroll = next(
                    (u for u in (8, 4, 2) if reps % u == 0), 1
                )
                with tc.For_i(0, reps // unroll, 1):
                    for _ in range(unroll):
                        emit_body(preload_next=True)

    nc.compile()
    return nc


def host_inputs(input, Wq, bq, Wk, bk, Wv, bv, drop_mask):
    """Build the per-core in_maps from the full problem inputs."""
    tiles_by_sc, MW = _mask_layout(S)
    idx = np.arange(128)
    # M[t, s] = 0 if t <= s else NEG ; matmul adds trilt[s, t] so ship M^T
    m_ts = np.where(idx[:, None] <= idx[None, :], 0.0, NEG).astype(np.float32)
    trilt = np.ascontiguousarray(m_ts.T).astype(ml_dtypes.bfloat16)
    shared = {
        "wqkv": np.ascontiguousarray(
            np.concatenate([Wq, Wk, Wv], axis=1).astype(np.float16)
        ),
        "b3": np.ascontiguousarray(
            np.stack(
                [np.asarray(bq), np.asarray(bk), np.asarray(bv)], axis=1
            ).astype(np.float32)
        ),
        "identb": np.eye(128, dtype=ml_dtypes.bfloat16),
        "identr": np.eye(128, dtype=np.float32),
        "trilt": trilt,
        "onesb": np.ones((128, 1), ml_dtypes.bfloat16),
    }
    in_maps = []
    for b in range(B):
        mt = np.asarray(drop_mask[b], np.float32).T  # [t, s]
        blocks = []
        for sc, row in enumerate(tiles_by_sc):
            for i, c0, n, off in row:
                scol = SCW * sc + c0
                blocks.append(mt[128 * i : 128 * (i + 1), scol : scol + n])
        maskp = np.ascontiguousarray(np.concatenate(blocks, axis=1)).astype(
            ml_dtypes.bfloat16
        )
        assert maskp.shape == (128, MW)
        in_maps.append(
            dict(
                shared,
                xt=np.ascontiguousarray(
                    np.asarray(input[b], np.float32).T.astype(np.float16)
                ),
                maskp=maskp,
            )
        )
    return in_maps


_NC_CACHE = {}


def get_nc(**kw):
    key = tuple(sorted(kw.items()))
    if key not in _NC_CACHE:
        _NC_CACHE[key] = build_nc(**kw)
    return _NC_CACHE[key]


def kernel(input, Wq, bq, Wk, bk, Wv, bv, drop_mask, **run_kwargs):
    nc = get_nc()
    in_maps = host_inputs(input, Wq, bq, Wk, bk, Wv, bv, drop_mask)
    res = run_bass_kernel_spmd(
        nc, in_maps, core_ids=list(range(NCORES)), **run_kwargs
    )
    out = np.stack([r["out"] for r in res.results]).astype(np.float32)
    if run_kwargs:
        kernel.last_result = res
    return out
